# revision 2
# baseline (speedup 1.0000x reference)
"""LoRA linear kernel for Trainium2 (8 NeuronCores, SPMD data-parallel).

Computes y = x @ (B @ A)^T for
    x: [4, 2048, 4096] f32, B: [4096, 16] f32, A: [16, 4096] f32.

Strategy: never materialize W = B @ A.  Factor as t = x @ A^T (rank 16)
then y = t @ B^T.  Tokens (4*2048 = 8192) are sharded across 8 cores
(1024 tokens each); A and B are replicated.

The kernel is HBM-DMA-bound; both streams are bf16 on the wire (x cast
on host, y upcast on host; rel err ~5e-3 vs the 2e-2 gate).

v2 schedule (from trace analysis of v1 @ 63.0us):
  - graded exec window = [first framework memset, last teardown inst];
    the ~6.3us init barrier is free, the ~8.7us teardown (255 sem
    resets emitted by the NEFF wrapper) is fixed.  The only lever is
    landing the LAST y write as early as possible.
  - one NC sustains ~425 GB/s on one HWDGE ring (fabric ceiling ~435,
    shared across rings), so ALL DMA rides the SP ring serially:
    at, bt, x chunks 0..7 (prefetched), then y writes as produced.
    Reads drain unimpeded by ~28.5us; writes follow back-to-back.
  - PSUM evacuation is the write-phase serializer in v1: a [128,512]
    f32-PSUM->bf16 copy is ~658ns on DVE (120+FD cyc @0.96GHz, 1x mode
    since PSUM src) and ~687ns on ACT; GPSIMD has no PSUM port.  v1's
    6:2 DVE:ACT split made DVE a 4us/chunk serial chain.  v2 splits
    4:4 (n%2) -> ~2.7us/chunk, and moves the y dma_start issue (~590ns)
    off the ACT queue onto the idle Sync queue so ACT only does evacs.
  - PE order: warms | oct g0 c0..c3 | tT0 | [mm2 g0 ck ; oct g1 ck] | tT1
    | mm2 g1 c0..c3.  mm2(g0,c0) is ready (tT0) before oct(g1,c0)'s
    chunk lands, so it goes FIRST; v1 had the octet ahead in the queue,
    blocking ready mm2 work ~3.4us and delaying all g0 writes to after
    the read phase.
  - HAM: PE clock starts at 4/8 (matmul 634ns vs 379ns); ~6 junk
    matmuls on a memset tile ramp it to 8/8 by the time chunk0 lands.
    In v1 the HAM re-throttled at 41.5us when the PE went idle-ish in
    the stretched write tail; v2's PE work ends ~37us so it never
    re-throttles on the critical path.
"""

import sys

import numpy as np

if "/opt/trn_rl_repo" not in sys.path:
    sys.path.insert(0, "/opt/trn_rl_repo")

# Problem shape (hardcoded per contract)
BATCH = 4
SEQ = 2048
D = 4096          # in_features == out_features
R = 16            # lora rank
NCORES = 8
NTOK = BATCH * SEQ            # 8192 tokens total
TOK = NTOK // NCORES          # 1024 tokens per core
P = 128                       # partitions
KO = D // P                   # 32 feature chunks
TB = 512                      # tokens per mm1 group (matmul free dim)
NG = TOK // TB                # 2 groups per core
NCHG = 4                      # x DMA chunks per group (1MB each)
KOC = KO // NCHG              # 8 ko-slices per chunk
NB = 512                      # matmul free dim for mm2 (psum bank limit)

# Module-level knobs for test.py (harness never touches these)
TRACE = False
LAST_RESULTS = None

_nc_cache = None


def _build_program():
    from concourse import bacc, mybir, tile

    # Bacc (not raw Bass): its finalize() runs generate_event_semaphores,
    # which splits multi-sem waits to satisfy TRN2's 1-wait-per-instruction
    # hardware constraint (walrus rejects >1 otherwise).
    nc = bacc.Bacc(
        "TRN2", target_bir_lowering=False, debug=False, num_devices=NCORES
    )

    f32 = mybir.dt.float32
    bf16 = mybir.dt.bfloat16

    xt = nc.dram_tensor("xt", [NG, NCHG, P, KOC, TB], bf16, kind="ExternalInput")
    at = nc.dram_tensor("at", [P, KO, R], bf16, kind="ExternalInput")
    bt = nc.dram_tensor("bt", [R, D], bf16, kind="ExternalInput")
    y = nc.dram_tensor("y", [TOK, D], bf16, kind="ExternalOutput")

    with tile.TileContext(nc) as tc:
        with (
            tc.tile_pool(name="consts", bufs=1) as consts,
            tc.tile_pool(name="xin", bufs=NG * NCHG) as xin,
            tc.tile_pool(name="tbuf", bufs=2) as tbuf,
            tc.tile_pool(name="yout", bufs=8) as yout,
            tc.tile_pool(name="pt", bufs=2, space="PSUM") as pt_pool,
            tc.tile_pool(name="py", bufs=6, space="PSUM") as py_pool,
        ):
            # Single SP (sync) HWDGE ring for everything; ring FIFO =
            # issue order: consts first, then all x chunks, then y writes.
            at_s = consts.tile([P, KO, R], bf16)
            nc.sync.dma_start(at_s[:], at[:])
            bt_s = consts.tile([R, D], bf16)
            nc.sync.dma_start(bt_s[:], bt[:])

            # HAM pre-warm: junk matmuls on a memset tile, gated only on
            # the memset, so the PE clock ramps to 8/8 during the DMA
            # prologue instead of during mm1 of the first group.
            junk = consts.tile([P, NB], bf16)
            nc.gpsimd.memset(junk[:], 0.0)

            def pe_warm(n):
                for _ in range(n):
                    warm = py_pool.tile([P, NB], f32, tag="psum_y")
                    nc.tensor.matmul(
                        warm[:], junk[:, :P], junk[:],
                        start=True, stop=True, skip_group_check=True,
                    )

            pe_warm(6)
            tc.no_sync_barrier()

            # Prefetch every x chunk up front (8MB; SBUF holds it all).
            xts = {}
            for g in range(NG):
                for c4 in range(NCHG):
                    t_ = xin.tile([P, KOC, TB], bf16, tag="xt")
                    nc.sync.dma_start(t_[:], xt[g, c4])
                    xts[(g, c4)] = t_

            def mm1_octet(g, c4, psum_t):
                # one 1MB fully-contiguous x chunk -> 8 accumulating matmuls
                xt_tile = xts[(g, c4)]
                for j in range(KOC):
                    ko = c4 * KOC + j
                    nc.tensor.matmul(
                        psum_t[:],
                        at_s[:, ko, :],
                        xt_tile[:, j, :],
                        start=(ko == 0),
                        stop=(ko == KO - 1),
                        skip_group_check=True,
                    )

            def make_tT(psum_t):
                # DVE copy psum f32 -> bf16 for the mm2 stationary operand
                tT = tbuf.tile([R, TB], bf16)
                nc.vector.tensor_copy(tT[:], psum_t[:])
                return tT

            def mm2_chunk(g, c, tT):
                y_row = yout.tile([P, D], bf16)
                for n in range(D // NB):
                    psum_y = py_pool.tile([P, NB], f32, tag="psum_y")
                    nc.tensor.matmul(
                        psum_y[:],
                        tT[:, c * P : (c + 1) * P],
                        bt_s[:, n * NB : (n + 1) * NB],
                        start=True,
                        stop=True,
                        skip_group_check=True,
                    )
                    # Single-bank PSUM evacuation, DVE 4 : ACT 4
                    if n % 2 == 0:
                        nc.vector.tensor_copy(y_row[:, n * NB : (n + 1) * NB], psum_y[:])
                    else:
                        nc.scalar.copy(y_row[:, n * NB : (n + 1) * NB], psum_y[:])
                row0 = g * TB + c * P
                # y writes issue from the (idle) Sync queue and queue on
                # the SP ring behind the reads.
                nc.sync.dma_start(y[row0 : row0 + P, :], y_row[:])

            # ---- schedule (see module docstring) ----
            psum_t0 = pt_pool.tile([R, TB], f32, tag="psum_t")
            for c4 in range(NCHG):
                mm1_octet(0, c4, psum_t0)
            tT0 = make_tT(psum_t0)

            psum_t1 = pt_pool.tile([R, TB], f32, tag="psum_t")
            for c4 in range(NCHG):
                mm2_chunk(0, c4, tT0)
                mm1_octet(1, c4, psum_t1)
            tT1 = make_tT(psum_t1)

            for c in range(NCHG):
                mm2_chunk(1, c, tT1)

    nc.finalize()
    return nc


def kernel(x, lora_matrix_B, lora_matrix_A):
    global _nc_cache, LAST_RESULTS
    import ml_dtypes
    from concourse.bass_utils import run_bass_kernel_spmd

    if _nc_cache is None:
        _nc_cache = _build_program()
    nc = _nc_cache

    bf16 = ml_dtypes.bfloat16
    x_flat = np.asarray(x, dtype=np.float32).reshape(NTOK, D).astype(bf16)
    A = np.asarray(lora_matrix_A, dtype=np.float32).astype(bf16)
    B = np.asarray(lora_matrix_B, dtype=np.float32).astype(bf16)

    # at[p, ko, j] = A[j, ko*128 + p];  bt[j, o] = B[o, j]
    at_prep = np.ascontiguousarray(A.reshape(R, KO, P).transpose(2, 1, 0))
    bt_prep = np.ascontiguousarray(B.T)

    in_maps = []
    for core in range(NCORES):
        xc = x_flat[core * TOK : (core + 1) * TOK, :]
        # xt[g, c4, p, j, t] = xc[g*512 + t, (c4*8 + j)*128 + p]
        xt_prep = np.ascontiguousarray(
            xc.reshape(NG, TB, NCHG, KOC, P).transpose(0, 2, 4, 3, 1)
        )
        in_maps.append({"xt": xt_prep, "at": at_prep, "bt": bt_prep})

    res = run_bass_kernel_spmd(
        nc, in_maps, core_ids=list(range(NCORES)), trace=TRACE
    )
    LAST_RESULTS = res

    y = np.concatenate([res.results[c]["y"] for c in range(NCORES)], axis=0)
    return y.reshape(BATCH, SEQ, D).astype(np.float32)
